# revision 41
# baseline (speedup 1.0000x reference)
"""ExternalAttention kernel for Trainium2 (8 NeuronCores, data-parallel on batch).

y = relu(x + Wv @ (l1norm_S(softmax_n(Wk @ x))))  per batch, with
x: [16, 512, 64, 64] f32, Wk: [8, 512], Wv: [512, 8].

Sharding: batch 16 -> 2 per core; Wk/Wv replicated. All softmax/L1 stats are
per (batch, s)/(batch, token), so fully local per core.

Matmuls run in fp32r (4x faster than fp32 on the PE for free dim >= 512);
x is rounded to fp32r during the load DMA (gpsimd casting DMA) and the
residual add/relu run in place on that tile, so the result carries only
the ~1.6e-4-relative fp32r roundings (~3.5e-4 of the output scale).
Cost-model (TimelineSim) per-core time: ~101 us vs a ~97 us DMA roofline
for the 32 MiB/core of traffic.
"""

import numpy as np

import concourse.bass as bass
import concourse.mybir as mybir
import concourse.tile as tile
from concourse import bacc
from concourse.bass_utils import run_bass_kernel_spmd
from concourse.masks import make_identity

F32 = mybir.dt.float32
F32R = mybir.dt.float32r

B, C, HH, WW = 16, 512, 64, 64
N = HH * WW          # 4096 tokens
S = 8                # attention "heads"/keys
NCORES = 8
BLOC = B // NCORES   # 2 batches per core
CCH = 128            # channel chunk == partition dim
NK = C // CCH        # 4 channel chunks
NCOL = 512           # matmul moving free dim (one PSUM bank of f32)
NJ = N // NCOL       # 8 column chunks
HALF = 1024          # x tile width (512 KiB DMA grain)
NH = N // HALF       # 2 halves
JPH = HALF // NCOL   # 4 column chunks per half
EPS = 1e-9


def build_nc(exact_x=False, inplace_out=True):
    nc = bacc.Bacc("TRN2")
    x = nc.dram_tensor("x", [BLOC, C, N], F32, kind="ExternalInput")
    wk = nc.dram_tensor("wk", [S, C], F32, kind="ExternalInput")
    wv = nc.dram_tensor("wv", [C, S], F32, kind="ExternalInput")
    y = nc.dram_tensor("y", [BLOC, C, N], F32, kind="ExternalOutput")

    mult = mybir.AluOpType.mult
    Exp = mybir.ActivationFunctionType.Exp
    Relu = mybir.ActivationFunctionType.Relu

    with tile.TileContext(nc) as tc:
        with (
            tc.tile_pool(name="const", bufs=1) as constp,
            tc.tile_pool(name="xt", bufs=2 * NK * NH) as xp,
            tc.tile_pool(name="u9", bufs=2) as up,
            tc.tile_pool(name="small", bufs=2) as sp,
            tc.tile_pool(name="cols", bufs=3) as cp,
            tc.tile_pool(name="xr", bufs=12) as xrp,
            tc.tile_pool(name="psE", bufs=2, space="PSUM") as psep,
            tc.tile_pool(name="psD", bufs=2, space="PSUM") as psdp,
            tc.tile_pool(name="psY", bufs=4, space="PSUM") as psyp,
        ):
            # --- constants -------------------------------------------------
            # Load weights with CONTIGUOUS descriptors (a transposing gather
            # DMA costs ~3.6us of 4-byte descriptors), transpose on the PE,
            # and round to f32r in the PSUM->SBUF copies.
            ident = constp.tile([CCH, CCH], F32)
            make_identity(nc, ident)
            # WkT[c, k, s] = Wk[s, 128k + c]; lhsT for matmul1 is WkT[:, k, :]
            wk_sb = constp.tile([S, C], F32)
            nc.sync.dma_start(out=wk_sb, in_=wk[:, :])
            wkT = constp.tile([CCH, NK, S], F32R)
            # plain-f32 twin for the hybrid first-row matmuls (see below)
            wkTf = constp.tile([CCH, NK, S], F32)
            for k in range(NK):
                pt = psyp.tile([CCH, S], F32, tag="psY")
                nc.tensor.transpose(
                    pt, in_=wk_sb[:, k * CCH:(k + 1) * CCH], identity=ident[0:S, 0:S]
                )
                nc.vector.tensor_copy(out=wkT[:, k, :], in_=pt)
                nc.scalar.copy(out=wkTf[:, k, :], in_=pt)
            # WvT[s, c] = Wv[c, s]; lhsT for matmul2 is WvT[:, k*128:...]
            wv_sb = constp.tile([CCH, NK, S], F32)
            for k in range(NK):
                nc.sync.dma_start(
                    out=wv_sb[:, k, :], in_=wv[k * CCH:(k + 1) * CCH, :]
                )
            wvT = constp.tile([S, C], F32R)
            for k in range(NK):
                pt = psyp.tile([S, CCH], F32, tag="psY")
                nc.tensor.transpose(pt, in_=wv_sb[:, k, :], identity=ident)
                nc.vector.tensor_copy(out=wvT[:, k * CCH:(k + 1) * CCH], in_=pt)
            ones8 = constp.tile([S, S], F32)
            nc.vector.memset(ones8, 1.0)
            # K=1 matmul operands that add EPS to every row of the denominator
            # (memset can't emit f32r, so stage f32 then round via copy)
            eps_lhs0 = constp.tile([1, S], F32)
            nc.vector.memset(eps_lhs0, EPS)
            eps_lhs = constp.tile([1, S], F32R)
            nc.vector.tensor_copy(out=eps_lhs, in_=eps_lhs0)
            one_row0 = constp.tile([1, NCOL], F32)
            nc.vector.memset(one_row0, 1.0)
            one_row = constp.tile([1, NCOL], F32R)
            nc.vector.tensor_copy(out=one_row, in_=one_row0)

            for b in range(BLOC):
                # --- load x (cast to fp32r in the DMA) ---------------------
                xt = {}
                for h in range(NH):
                    for k in range(NK):
                        # hybrid start: the very first tile-row goes through
                        # HWDGE (sync) as plain f32 — it skips the ~0.6us
                        # SWDGE Q7 descriptor-gen latency, so the DMA stream
                        # starts sooner; its 2 matmul columns run fp32.
                        first_row = (b == 0 and h == 0 and not exact_x)
                        t = xp.tile(
                            [CCH, HALF],
                            F32 if (exact_x or first_row) else F32R,
                            tag="xt",
                        )
                        eng = nc.sync if first_row else nc.gpsimd
                        eng.dma_start(
                            out=t,
                            in_=x[b, k * CCH:(k + 1) * CCH, h * HALF:(h + 1) * HALF],
                        )
                        xt[k, h] = t

                # --- E = Wk @ x, U = exp(E), Z = sum_n U -------------------
                u9 = up.tile([S, N], F32R, tag="u9")
                zp_t = sp.tile([S, NJ], F32, tag="zp")
                for j in range(NJ):
                    h, jc0 = divmod(j * NCOL, HALF)
                    psE = psep.tile([S, NCOL], F32, tag="psE")
                    for k in range(NK):
                        first_row = (b == 0 and h == 0 and not exact_x)
                        if exact_x:
                            # JIT-round x to f32r on ACT for the matmul only;
                            # the residual add keeps the exact f32 x
                            xr = xrp.tile([CCH, NCOL], F32R, tag="xr")
                            nc.scalar.copy(out=xr, in_=xt[k, h][:, jc0:jc0 + NCOL])
                            rhs = xr
                        else:
                            rhs = xt[k, h][:, jc0:jc0 + NCOL]
                        nc.tensor.matmul(
                            psE,
                            lhsT=wkTf[:, k, :] if first_row else wkT[:, k, :],
                            rhs=rhs,
                            start=(k == 0),
                            stop=(k == NK - 1),
                        )
                    nc.scalar.activation(
                        out=u9[:, j * NCOL:(j + 1) * NCOL],
                        in_=psE,
                        func=Exp,
                        accum_out=zp_t[:, j:j + 1],
                    )

                z_t = sp.tile([S, 1], F32, tag="z")
                nc.vector.reduce_sum(out=z_t, in_=zp_t, axis=mybir.AxisListType.X)
                zinv = sp.tile([S, 1], F32, tag="zinv")
                nc.vector.reciprocal(out=zinv, in_=z_t)

                # zlhs rows = zinv[s] broadcast over 8 cols, so
                #   (zlhs.T @ U)[m, n] = sum_s zinv[s] U[s, n]   for all m,
                # then a K=1 matmul with (eps_lhs, one_row) accumulates +EPS.
                zlhs = sp.tile([S, S], F32R, tag="zlhs")
                nc.vector.tensor_scalar_mul(out=zlhs, in0=ones8, scalar1=zinv)

                for j in range(NJ):
                    jc = slice(j * NCOL, (j + 1) * NCOL)
                    h, jc0 = divmod(j * NCOL, HALF)
                    psD = psdp.tile([S, NCOL], F32, tag="psD")
                    nc.tensor.matmul(psD, lhsT=zlhs, rhs=u9[:, jc], start=True, stop=False)
                    nc.tensor.matmul(psD, lhsT=eps_lhs, rhs=one_row, start=False, stop=True)
                    rD = cp.tile([S, NCOL], F32, tag="rD")
                    nc.vector.reciprocal(out=rD, in_=psD)
                    # a2 = (U * zinv) * (1 / denom)
                    a2 = cp.tile([S, NCOL], F32R, tag="a2")
                    nc.vector.scalar_tensor_tensor(
                        out=a2, in0=u9[:, jc], scalar=zinv, in1=rD, op0=mult, op1=mult
                    )
                    for k in range(NK):
                        psY = psyp.tile([CCH, NCOL], F32, tag="psY")
                        nc.tensor.matmul(
                            psY,
                            lhsT=wvT[:, k * CCH:(k + 1) * CCH],
                            rhs=a2,
                            start=True,
                            stop=True,
                        )
                        first_row = (b == 0 and h == 0 and not exact_x)
                        xv = xt[k, h][:, jc0:jc0 + NCOL]
                        if inplace_out:
                            nc.vector.tensor_add(out=xv, in0=xv, in1=psY)
                            nc.scalar.activation(out=xv, in_=xv, func=Relu)
                            src = xv if (exact_x or first_row) else xv.bitcast(F32)
                        else:
                            # exact-f32 epilogue into a separate column tile:
                            # only x itself carries the f32r load rounding
                            ycol = xrp.tile([CCH, NCOL], F32, tag="ycol")
                            nc.vector.tensor_add(
                                out=ycol, in0=xv if exact_x else xv.bitcast(F32), in1=psY
                            )
                            nc.scalar.activation(out=ycol, in_=ycol, func=Relu)
                            src = ycol
                        nc.sync.dma_start(
                            out=y[b, k * CCH:(k + 1) * CCH, jc],
                            in_=src,
                        )

    nc.finalize()
    return nc


_NC_CACHE = None


def _get_nc():
    global _NC_CACHE
    if _NC_CACHE is None:
        _NC_CACHE = build_nc()
    return _NC_CACHE


def kernel(x, Wk, Wv):
    x = np.ascontiguousarray(np.asarray(x, dtype=np.float32))
    Wk = np.ascontiguousarray(np.asarray(Wk, dtype=np.float32))
    Wv = np.ascontiguousarray(np.asarray(Wv, dtype=np.float32))
    assert x.shape == (B, C, HH, WW), x.shape
    xr = x.reshape(B, C, N)

    nc = _get_nc()
    in_maps = [
        {"x": xr[i * BLOC:(i + 1) * BLOC], "wk": Wk, "wv": Wv}
        for i in range(NCORES)
    ]
    res = run_bass_kernel_spmd(nc, in_maps, list(range(NCORES)))
    out = np.concatenate([res.results[i]["y"] for i in range(NCORES)], axis=0)
    return out.reshape(B, C, HH, WW)



# revision 42
# speedup vs baseline: 1.0049x; 1.0049x over previous
"""ExternalAttention kernel for Trainium2 (8 NeuronCores, data-parallel on batch).

y = relu(x + Wv @ (l1norm_S(softmax_n(Wk @ x))))  per batch, with
x: [16, 512, 64, 64] f32, Wk: [8, 512], Wv: [512, 8].

Sharding: batch 16 -> 2 per core; Wk/Wv replicated. All softmax/L1 stats are
per (batch, s)/(batch, token), so fully local per core.

Matmuls run in fp32r (4x faster than fp32 on the PE for free dim >= 512);
x is rounded to fp32r during the load DMA (gpsimd casting DMA) and the
residual add/relu run in place on that tile, so the result carries only
the ~1.6e-4-relative fp32r roundings (~3.5e-4 of the output scale).
Cost-model (TimelineSim) per-core time: ~101 us vs a ~97 us DMA roofline
for the 32 MiB/core of traffic.
"""

import numpy as np

import concourse.bass as bass
import concourse.mybir as mybir
import concourse.tile as tile
from concourse import bacc
from concourse.bass_utils import run_bass_kernel_spmd
from concourse.masks import make_identity

F32 = mybir.dt.float32
F32R = mybir.dt.float32r

B, C, HH, WW = 16, 512, 64, 64
N = HH * WW          # 4096 tokens
S = 8                # attention "heads"/keys
NCORES = 8
BLOC = B // NCORES   # 2 batches per core
CCH = 128            # channel chunk == partition dim
NK = C // CCH        # 4 channel chunks
NCOL = 512           # matmul moving free dim (one PSUM bank of f32)
NJ = N // NCOL       # 8 column chunks
HALF = 1024          # x tile width (512 KiB DMA grain)
NH = N // HALF       # 2 halves
JPH = HALF // NCOL   # 4 column chunks per half
EPS = 1e-9


def build_nc(exact_x=False, inplace_out=True):
    nc = bacc.Bacc("TRN2")
    x = nc.dram_tensor("x", [BLOC, C, N], F32, kind="ExternalInput")
    wk = nc.dram_tensor("wk", [S, C], F32, kind="ExternalInput")
    wv = nc.dram_tensor("wv", [C, S], F32, kind="ExternalInput")
    y = nc.dram_tensor("y", [BLOC, C, N], F32, kind="ExternalOutput")

    mult = mybir.AluOpType.mult
    Exp = mybir.ActivationFunctionType.Exp
    Relu = mybir.ActivationFunctionType.Relu

    with tile.TileContext(nc) as tc:
        with (
            tc.tile_pool(name="const", bufs=1) as constp,
            tc.tile_pool(name="xt", bufs=2 * NK * NH) as xp,
            tc.tile_pool(name="u9", bufs=2) as up,
            tc.tile_pool(name="small", bufs=2) as sp,
            tc.tile_pool(name="cols", bufs=3) as cp,
            tc.tile_pool(name="xr", bufs=12) as xrp,
            tc.tile_pool(name="psE", bufs=2, space="PSUM") as psep,
            tc.tile_pool(name="psD", bufs=2, space="PSUM") as psdp,
            tc.tile_pool(name="psY", bufs=4, space="PSUM") as psyp,
        ):
            # --- constants -------------------------------------------------
            # Load weights with CONTIGUOUS descriptors (a transposing gather
            # DMA costs ~3.6us of 4-byte descriptors), transpose on the PE,
            # and round to f32r in the PSUM->SBUF copies.
            ident = constp.tile([CCH, CCH], F32)
            make_identity(nc, ident)
            # WkT[c, k, s] = Wk[s, 128k + c]; lhsT for matmul1 is WkT[:, k, :]
            wk_sb = constp.tile([S, C], F32)
            nc.sync.dma_start(out=wk_sb, in_=wk[:, :])
            wkT = constp.tile([CCH, NK, S], F32R)
            for k in range(NK):
                pt = psyp.tile([CCH, S], F32, tag="psY")
                nc.tensor.transpose(
                    pt, in_=wk_sb[:, k * CCH:(k + 1) * CCH], identity=ident[0:S, 0:S]
                )
                nc.vector.tensor_copy(out=wkT[:, k, :], in_=pt)
            # WvT[s, c] = Wv[c, s]; lhsT for matmul2 is WvT[:, k*128:...]
            wv_sb = constp.tile([CCH, NK, S], F32)
            for k in range(NK):
                nc.sync.dma_start(
                    out=wv_sb[:, k, :], in_=wv[k * CCH:(k + 1) * CCH, :]
                )
            wvT = constp.tile([S, C], F32R)
            for k in range(NK):
                pt = psyp.tile([S, CCH], F32, tag="psY")
                nc.tensor.transpose(pt, in_=wv_sb[:, k, :], identity=ident)
                nc.vector.tensor_copy(out=wvT[:, k * CCH:(k + 1) * CCH], in_=pt)
            ones8 = constp.tile([S, S], F32)
            nc.vector.memset(ones8, 1.0)
            # K=1 matmul operands that add EPS to every row of the denominator
            # (memset can't emit f32r, so stage f32 then round via copy)
            eps_lhs0 = constp.tile([1, S], F32)
            nc.vector.memset(eps_lhs0, EPS)
            eps_lhs = constp.tile([1, S], F32R)
            nc.vector.tensor_copy(out=eps_lhs, in_=eps_lhs0)
            one_row0 = constp.tile([1, NCOL], F32)
            nc.vector.memset(one_row0, 1.0)
            one_row = constp.tile([1, NCOL], F32R)
            nc.vector.tensor_copy(out=one_row, in_=one_row0)

            for b in range(BLOC):
                # --- load x (cast to fp32r in the DMA) ---------------------
                xt = {}
                for h in range(NH):
                    for k in range(NK):
                        t = xp.tile([CCH, HALF], F32 if exact_x else F32R, tag="xt")
                        nc.gpsimd.dma_start(
                            out=t,
                            in_=x[b, k * CCH:(k + 1) * CCH, h * HALF:(h + 1) * HALF],
                        )
                        xt[k, h] = t

                # --- E = Wk @ x, U = exp(E), Z = sum_n U -------------------
                u9 = up.tile([S, N], F32R, tag="u9")
                zp_t = sp.tile([S, NJ], F32, tag="zp")
                for j in range(NJ):
                    h, jc0 = divmod(j * NCOL, HALF)
                    psE = psep.tile([S, NCOL], F32, tag="psE")
                    for k in range(NK):
                        if exact_x:
                            # JIT-round x to f32r on ACT for the matmul only;
                            # the residual add keeps the exact f32 x
                            xr = xrp.tile([CCH, NCOL], F32R, tag="xr")
                            nc.scalar.copy(out=xr, in_=xt[k, h][:, jc0:jc0 + NCOL])
                            rhs = xr
                        else:
                            rhs = xt[k, h][:, jc0:jc0 + NCOL]
                        nc.tensor.matmul(
                            psE,
                            lhsT=wkT[:, k, :],
                            rhs=rhs,
                            start=(k == 0),
                            stop=(k == NK - 1),
                        )
                    nc.scalar.activation(
                        out=u9[:, j * NCOL:(j + 1) * NCOL],
                        in_=psE,
                        func=Exp,
                        accum_out=zp_t[:, j:j + 1],
                    )

                z_t = sp.tile([S, 1], F32, tag="z")
                nc.vector.reduce_sum(out=z_t, in_=zp_t, axis=mybir.AxisListType.X)
                zinv = sp.tile([S, 1], F32, tag="zinv")
                nc.vector.reciprocal(out=zinv, in_=z_t)

                # zlhs rows = zinv[s] broadcast over 8 cols, so
                #   (zlhs.T @ U)[m, n] = sum_s zinv[s] U[s, n]   for all m,
                # then a K=1 matmul with (eps_lhs, one_row) accumulates +EPS.
                zlhs = sp.tile([S, S], F32R, tag="zlhs")
                nc.vector.tensor_scalar_mul(out=zlhs, in0=ones8, scalar1=zinv)

                for j in range(NJ):
                    jc = slice(j * NCOL, (j + 1) * NCOL)
                    h, jc0 = divmod(j * NCOL, HALF)
                    psD = psdp.tile([S, NCOL], F32, tag="psD")
                    nc.tensor.matmul(psD, lhsT=zlhs, rhs=u9[:, jc], start=True, stop=False)
                    nc.tensor.matmul(psD, lhsT=eps_lhs, rhs=one_row, start=False, stop=True)
                    rD = cp.tile([S, NCOL], F32, tag="rD")
                    nc.vector.reciprocal(out=rD, in_=psD)
                    # a2 = (U * zinv) * (1 / denom)
                    a2 = cp.tile([S, NCOL], F32R, tag="a2")
                    nc.vector.scalar_tensor_tensor(
                        out=a2, in0=u9[:, jc], scalar=zinv, in1=rD, op0=mult, op1=mult
                    )
                    for k in range(NK):
                        psY = psyp.tile([CCH, NCOL], F32, tag="psY")
                        nc.tensor.matmul(
                            psY,
                            lhsT=wvT[:, k * CCH:(k + 1) * CCH],
                            rhs=a2,
                            start=True,
                            stop=True,
                        )
                        xv = xt[k, h][:, jc0:jc0 + NCOL]
                        if inplace_out:
                            nc.vector.tensor_add(out=xv, in0=xv, in1=psY)
                            nc.scalar.activation(out=xv, in_=xv, func=Relu)
                            src = xv if exact_x else xv.bitcast(F32)
                        else:
                            # exact-f32 epilogue into a separate column tile:
                            # only x itself carries the f32r load rounding
                            ycol = xrp.tile([CCH, NCOL], F32, tag="ycol")
                            nc.vector.tensor_add(
                                out=ycol, in0=xv if exact_x else xv.bitcast(F32), in1=psY
                            )
                            nc.scalar.activation(out=ycol, in_=ycol, func=Relu)
                            src = ycol
                        nc.sync.dma_start(
                            out=y[b, k * CCH:(k + 1) * CCH, jc],
                            in_=src,
                        )

    nc.finalize()
    return nc


_NC_CACHE = None


def _get_nc():
    global _NC_CACHE
    if _NC_CACHE is None:
        _NC_CACHE = build_nc()
    return _NC_CACHE


def kernel(x, Wk, Wv):
    x = np.ascontiguousarray(np.asarray(x, dtype=np.float32))
    Wk = np.ascontiguousarray(np.asarray(Wk, dtype=np.float32))
    Wv = np.ascontiguousarray(np.asarray(Wv, dtype=np.float32))
    assert x.shape == (B, C, HH, WW), x.shape
    xr = x.reshape(B, C, N)

    nc = _get_nc()
    in_maps = [
        {"x": xr[i * BLOC:(i + 1) * BLOC], "wk": Wk, "wv": Wv}
        for i in range(NCORES)
    ]
    res = run_bass_kernel_spmd(nc, in_maps, list(range(NCORES)))
    out = np.concatenate([res.results[i]["y"] for i in range(NCORES)], axis=0)
    return out.reshape(B, C, HH, WW)



# revision 43
# speedup vs baseline: 1.0132x; 1.0082x over previous
"""ExternalAttention kernel for Trainium2 (8 NeuronCores, data-parallel on batch).

y = relu(x + Wv @ (l1norm_S(softmax_n(Wk @ x))))  per batch, with
x: [16, 512, 64, 64] f32, Wk: [8, 512], Wv: [512, 8].

Sharding: batch 16 -> 2 per core; Wk/Wv replicated. All softmax/L1 stats are
per (batch, s)/(batch, token), so fully local per core.

Matmuls run in fp32r (4x faster than fp32 on the PE for free dim >= 512);
x is rounded to fp32r during the load DMA (gpsimd casting DMA) and the
residual add/relu run in place on that tile, so the result carries only
the ~1.6e-4-relative fp32r roundings (~3.5e-4 of the output scale).
Cost-model (TimelineSim) per-core time: ~101 us vs a ~97 us DMA roofline
for the 32 MiB/core of traffic.
"""

import numpy as np

import concourse.bass as bass
import concourse.mybir as mybir
import concourse.tile as tile
from concourse import bacc
from concourse.bass_utils import run_bass_kernel_spmd
from concourse.masks import make_identity

F32 = mybir.dt.float32
F32R = mybir.dt.float32r

B, C, HH, WW = 16, 512, 64, 64
N = HH * WW          # 4096 tokens
S = 8                # attention "heads"/keys
NCORES = 8
BLOC = B // NCORES   # 2 batches per core
CCH = 128            # channel chunk == partition dim
NK = C // CCH        # 4 channel chunks
NCOL = 512           # matmul moving free dim (one PSUM bank of f32)
NJ = N // NCOL       # 8 column chunks
HALF = 1024          # x tile width (512 KiB DMA grain)
NH = N // HALF       # 2 halves
JPH = HALF // NCOL   # 4 column chunks per half
EPS = 1e-9


def build_nc(exact_x=False, inplace_out=True):
    nc = bacc.Bacc("TRN2")
    x = nc.dram_tensor("x", [BLOC, C, N], F32, kind="ExternalInput")
    wk = nc.dram_tensor("wk", [S, C], F32, kind="ExternalInput")
    wv = nc.dram_tensor("wv", [C, S], F32, kind="ExternalInput")
    y = nc.dram_tensor("y", [BLOC, C, N], F32, kind="ExternalOutput")

    mult = mybir.AluOpType.mult
    Exp = mybir.ActivationFunctionType.Exp
    Relu = mybir.ActivationFunctionType.Relu

    with tile.TileContext(nc) as tc:
        with (
            tc.tile_pool(name="const", bufs=1) as constp,
            tc.tile_pool(name="xt", bufs=2 * NK * NH) as xp,
            tc.tile_pool(name="u9", bufs=2) as up,
            tc.tile_pool(name="small", bufs=2) as sp,
            tc.tile_pool(name="cols", bufs=3) as cp,
            tc.tile_pool(name="xr", bufs=12) as xrp,
            tc.tile_pool(name="psE", bufs=2, space="PSUM") as psep,
            tc.tile_pool(name="psD", bufs=2, space="PSUM") as psdp,
            tc.tile_pool(name="psY", bufs=4, space="PSUM") as psyp,
        ):
            # --- constants -------------------------------------------------
            # Load weights with CONTIGUOUS descriptors (a transposing gather
            # DMA costs ~3.6us of 4-byte descriptors), transpose on the PE,
            # and round to f32r in the PSUM->SBUF copies.
            ident = constp.tile([CCH, CCH], F32)
            make_identity(nc, ident)
            identR = constp.tile([CCH, CCH], F32R)
            nc.vector.tensor_copy(out=identR, in_=ident)
            # WkT[c, k, s] = Wk[s, 128k + c]; lhsT for matmul1 is WkT[:, k, :]
            wk_sb = constp.tile([S, C], F32)
            nc.sync.dma_start(out=wk_sb, in_=wk[:, :])
            wkT = constp.tile([CCH, NK, S], F32R)
            for k in range(NK):
                pt = psyp.tile([CCH, S], F32, tag="psY")
                nc.tensor.transpose(
                    pt, in_=wk_sb[:, k * CCH:(k + 1) * CCH], identity=ident[0:S, 0:S]
                )
                nc.vector.tensor_copy(out=wkT[:, k, :], in_=pt)
            # WvT[s, c] = Wv[c, s]; lhsT for matmul2 is WvT[:, k*128:...]
            wv_sb = constp.tile([CCH, NK, S], F32)
            for k in range(NK):
                nc.sync.dma_start(
                    out=wv_sb[:, k, :], in_=wv[k * CCH:(k + 1) * CCH, :]
                )
            wvT = constp.tile([S, C], F32R)
            for k in range(NK):
                pt = psyp.tile([S, CCH], F32, tag="psY")
                nc.tensor.transpose(pt, in_=wv_sb[:, k, :], identity=ident)
                nc.vector.tensor_copy(out=wvT[:, k * CCH:(k + 1) * CCH], in_=pt)
            ones8 = constp.tile([S, S], F32)
            nc.vector.memset(ones8, 1.0)
            # K=1 matmul operands that add EPS to every row of the denominator
            # (memset can't emit f32r, so stage f32 then round via copy)
            eps_lhs0 = constp.tile([1, S], F32)
            nc.vector.memset(eps_lhs0, EPS)
            eps_lhs = constp.tile([1, S], F32R)
            nc.vector.tensor_copy(out=eps_lhs, in_=eps_lhs0)
            one_row0 = constp.tile([1, NCOL], F32)
            nc.vector.memset(one_row0, 1.0)
            one_row = constp.tile([1, NCOL], F32R)
            nc.vector.tensor_copy(out=one_row, in_=one_row0)

            for b in range(BLOC):
                # --- load x (cast to fp32r in the DMA) ---------------------
                xt = {}
                for h in range(NH):
                    for k in range(NK):
                        t = xp.tile([CCH, HALF], F32 if exact_x else F32R, tag="xt")
                        nc.gpsimd.dma_start(
                            out=t,
                            in_=x[b, k * CCH:(k + 1) * CCH, h * HALF:(h + 1) * HALF],
                        )
                        xt[k, h] = t

                # --- E = Wk @ x, U = exp(E), Z = sum_n U -------------------
                u9 = up.tile([S, N], F32R, tag="u9")
                zp_t = sp.tile([S, NJ], F32, tag="zp")
                for j in range(NJ):
                    h, jc0 = divmod(j * NCOL, HALF)
                    psE = psep.tile([S, NCOL], F32, tag="psE")
                    for k in range(NK):
                        if exact_x:
                            # JIT-round x to f32r on ACT for the matmul only;
                            # the residual add keeps the exact f32 x
                            xr = xrp.tile([CCH, NCOL], F32R, tag="xr")
                            nc.scalar.copy(out=xr, in_=xt[k, h][:, jc0:jc0 + NCOL])
                            rhs = xr
                        else:
                            rhs = xt[k, h][:, jc0:jc0 + NCOL]
                        nc.tensor.matmul(
                            psE,
                            lhsT=wkT[:, k, :],
                            rhs=rhs,
                            start=(k == 0),
                            stop=(k == NK - 1),
                        )
                    nc.scalar.activation(
                        out=u9[:, j * NCOL:(j + 1) * NCOL],
                        in_=psE,
                        func=Exp,
                        accum_out=zp_t[:, j:j + 1],
                    )

                z_t = sp.tile([S, 1], F32, tag="z")
                nc.vector.reduce_sum(out=z_t, in_=zp_t, axis=mybir.AxisListType.X)
                zinv = sp.tile([S, 1], F32, tag="zinv")
                nc.vector.reciprocal(out=zinv, in_=z_t)

                # zlhs rows = zinv[s] broadcast over 8 cols, so
                #   (zlhs.T @ U)[m, n] = sum_s zinv[s] U[s, n]   for all m,
                # then a K=1 matmul with (eps_lhs, one_row) accumulates +EPS.
                zlhs = sp.tile([S, S], F32R, tag="zlhs")
                nc.vector.tensor_scalar_mul(out=zlhs, in0=ones8, scalar1=zinv)

                for j in range(NJ):
                    jc = slice(j * NCOL, (j + 1) * NCOL)
                    h, jc0 = divmod(j * NCOL, HALF)
                    psD = psdp.tile([S, NCOL], F32, tag="psD")
                    nc.tensor.matmul(psD, lhsT=zlhs, rhs=u9[:, jc], start=True, stop=False)
                    nc.tensor.matmul(psD, lhsT=eps_lhs, rhs=one_row, start=False, stop=True)
                    rD = cp.tile([S, NCOL], F32, tag="rD")
                    nc.vector.reciprocal(out=rD, in_=psD)
                    # a2 = (U * zinv) * (1 / denom)
                    a2 = cp.tile([S, NCOL], F32R, tag="a2")
                    nc.vector.scalar_tensor_tensor(
                        out=a2, in0=u9[:, jc], scalar=zinv, in1=rD, op0=mult, op1=mult
                    )
                    for k in range(NK):
                        psY = psyp.tile([CCH, NCOL], F32, tag="psY")
                        xv = xt[k, h][:, jc0:jc0 + NCOL]
                        if inplace_out and not exact_x:
                            # residual add on the PE: psY = I.T@x + Wv@a2,
                            # then relu drains PSUM back into the x tile.
                            nc.tensor.matmul(
                                psY, lhsT=identR, rhs=xv, start=True, stop=False
                            )
                            nc.tensor.matmul(
                                psY,
                                lhsT=wvT[:, k * CCH:(k + 1) * CCH],
                                rhs=a2,
                                start=False,
                                stop=True,
                            )
                            nc.scalar.activation(out=xv, in_=psY, func=Relu)
                            nc.sync.dma_start(
                                out=y[b, k * CCH:(k + 1) * CCH, jc],
                                in_=xv.bitcast(F32),
                            )
                            continue
                        nc.tensor.matmul(
                            psY,
                            lhsT=wvT[:, k * CCH:(k + 1) * CCH],
                            rhs=a2,
                            start=True,
                            stop=True,
                        )
                        if inplace_out:
                            nc.vector.tensor_add(out=xv, in0=xv, in1=psY)
                            nc.scalar.activation(out=xv, in_=xv, func=Relu)
                            src = xv if exact_x else xv.bitcast(F32)
                        else:
                            # exact-f32 epilogue into a separate column tile:
                            # only x itself carries the f32r load rounding
                            ycol = xrp.tile([CCH, NCOL], F32, tag="ycol")
                            nc.vector.tensor_add(
                                out=ycol, in0=xv if exact_x else xv.bitcast(F32), in1=psY
                            )
                            nc.scalar.activation(out=ycol, in_=ycol, func=Relu)
                            src = ycol
                        nc.sync.dma_start(
                            out=y[b, k * CCH:(k + 1) * CCH, jc],
                            in_=src,
                        )

    nc.finalize()
    return nc


_NC_CACHE = None


def _get_nc():
    global _NC_CACHE
    if _NC_CACHE is None:
        _NC_CACHE = build_nc()
    return _NC_CACHE


def kernel(x, Wk, Wv):
    x = np.ascontiguousarray(np.asarray(x, dtype=np.float32))
    Wk = np.ascontiguousarray(np.asarray(Wk, dtype=np.float32))
    Wv = np.ascontiguousarray(np.asarray(Wv, dtype=np.float32))
    assert x.shape == (B, C, HH, WW), x.shape
    xr = x.reshape(B, C, N)

    nc = _get_nc()
    in_maps = [
        {"x": xr[i * BLOC:(i + 1) * BLOC], "wk": Wk, "wv": Wv}
        for i in range(NCORES)
    ]
    res = run_bass_kernel_spmd(nc, in_maps, list(range(NCORES)))
    out = np.concatenate([res.results[i]["y"] for i in range(NCORES)], axis=0)
    return out.reshape(B, C, HH, WW)

